# revision 48
# baseline (speedup 1.0000x reference)
"""Multi-head attention (B=2, S=2048, D=1024, H=16, Dh=64) on 8 Trainium2
NeuronCores via Bass/Tile.

Sharding: data-parallel over the 2 batches x tensor-parallel over head
groups (16 heads -> 4 groups of 4). Core c = 4*b + g handles batch b and
heads 4g..4g+3 with the matching column/row slices of Wq/Wk/Wv/Wo. Each
core returns its partial output projection; the host sums the 4 partials
per batch and adds bo' (bo' = bo + bv @ Wo, so the V bias never has to be
applied on device: softmax weights sum to 1, so ctx = sum_k p_k (xWv)_k
+ bv, and the bv term commutes through Wo).

Host passes x pre-transposed (xT, [D, S]) so the kernel needs no on-chip
transposes. Per-core kernel (4 heads = 2 pairs of 64-dim heads stacked on
the 128-partition dim), bf16 matmul datapath with fp32 PSUM accumulation:
  QT   = Wq_g^T x^T + bq_g              [128 (2 heads x 64), 2 pairs, S]
  KT   = Wk_g^T x^T + bk_g              (same layout)
  V_ext= [(x Wv_g) * maskf | maskf]     [s, chunk, 4*(64+1)] bf16
  per pair, per q-tile (512 queries), per key chunk (128 keys):
    scT [128k, 2x512q] = KT_chunk^T @ QT_tile   (2 heads row-packed in PE)
    eT  = exp(SCALE * scT)                      (one ACT op per pair, bf16)
    ctx_h[65, 512] += V_ext_chunk^T @ eT_h      (row 64 = softmax denom)
  normalize: recip(den) -> broadcast (GPSIMD) -> ctxT = ctx*rec
  out_partial = ctxT^T @ Wo_g           (PSUM accum over the 2 pairs)

The masked-softmax trick: exp is taken over unmasked scores (safe: |score*
SCALE| < ~3 here), and the 0/1 key mask is folded into V_ext (zeroed V rows
and the mask column), so masked keys contribute 0 to both the numerator and
the denominator -- no -inf arithmetic on device.

Emission order software-pipelines the ACT (exp) engine: QK projections for
pair 0 are chased by attention(0, qt) so the first exp issues ~6us in, and
V projections / pair-1 projections / output projections fill PE slack while
ACT streams exp chunks back-to-back (ACT is the bottleneck engine at ~147us
busy; total PE work is ~137us).
"""

import ml_dtypes
import numpy as np

import concourse.bacc as bacc
import concourse.mybir as mybir
import concourse.tile as tile
from concourse.bass_utils import run_bass_kernel_spmd

F32 = mybir.dt.float32
BF16 = mybir.dt.bfloat16
AF = mybir.ActivationFunctionType

S = 2048
D = 1024
HPC = 4                  # heads per core
DH = 64
PAIRS = 2                # head pairs per core
P = 128
SC_CHUNKS = S // P       # 16 key chunks
QT_TILES = 4             # q tiles of 512
QW = 512                 # q tile width
ST_TILES = S // P        # 16 s tiles
DCH = D // P             # 8 D chunks
SCALE = 1.0 / np.sqrt(DH)
CTX_LAG = 3

N_CORES = 8


def build():
    nc = bacc.Bacc(None, target_bir_lowering=False, num_swdge_queues=4)

    # All inputs arrive host-pre-tiled in their exact SBUF layouts (bf16 where
    # the datapath is bf16) so every load DMA reads big contiguous lines.
    xt = nc.dram_tensor("xt", [P, QT_TILES, DCH, QW], BF16, kind="ExternalInput")
    wq0 = nc.dram_tensor("wq0", [P, DCH, P], BF16, kind="ExternalInput")
    wq1 = nc.dram_tensor("wq1", [P, DCH, P], BF16, kind="ExternalInput")
    wk0 = nc.dram_tensor("wk0", [P, DCH, P], BF16, kind="ExternalInput")
    wk1 = nc.dram_tensor("wk1", [P, DCH, P], BF16, kind="ExternalInput")
    wv = nc.dram_tensor("wv", [P, DCH, 256], BF16, kind="ExternalInput")
    wo = nc.dram_tensor("wo", [P, PAIRS, D], BF16, kind="ExternalInput")
    bq = nc.dram_tensor("bq", [P, PAIRS], F32, kind="ExternalInput")
    bk = nc.dram_tensor("bk", [P, PAIRS], F32, kind="ExternalInput")
    maskf = nc.dram_tensor("maskf", [P, SC_CHUNKS], F32, kind="ExternalInput")
    out = nc.dram_tensor("out", [S, D], BF16, kind="ExternalOutput")

    with tile.TileContext(nc) as tc:
        with (
            tc.tile_pool(name="persist", bufs=1) as pp,
            tc.tile_pool(name="expp", bufs=8) as ep,
            tc.tile_pool(name="ostage", bufs=2) as op_,
            tc.tile_pool(name="smalls", bufs=3) as sp,
            tc.tile_pool(name="ps_sc", bufs=2, space="PSUM") as ps_sc,
            tc.tile_pool(name="ps_ctx", bufs=2, space="PSUM") as ps_ctx,
            tc.tile_pool(name="ps_w", bufs=2, space="PSUM") as ps_w,
        ):
            # ---- constants / persistent tensors ----
            # weights first: qk_proj(0, 0) needs wq/wk asap
            wq_sb = pp.tile([P, DCH, 256], BF16)
            wk_sb = pp.tile([P, DCH, 256], BF16)
            wv_sb = pp.tile([P, DCH, 256], BF16)
            wo_sb = pp.tile([P, PAIRS, D], BF16)
            # casting DMAs (fp32 DRAM -> bf16 SBUF) must go via gpsimd/SWDGE
            # x slices stream on the HWDGE (sync) queue, weights on the SWDGE
            # (gpsimd) queues, so the qk chase and V projections are never
            # serialized behind each other's loads.
            bq_sb = pp.tile([P, PAIRS], F32)
            bk_sb = pp.tile([P, PAIRS], F32)
            maskp = pp.tile([P, SC_CHUNKS], F32)
            nc.sync.dma_start(maskp[:], maskf[:])
            nc.sync.dma_start(bq_sb[:], bq[:])
            nc.sync.dma_start(bk_sb[:], bk[:])
            # warm the ACT exp table while DMAs stream (table load ~2.7us)
            actwarm = sp.tile([P, PAIRS], F32, tag="actwarm")
            nc.scalar.activation(actwarm[:], bq_sb[:], AF.Exp, scale=1.0)
            # warm the ACT exp table while DMAs stream (table load ~2.7us)
            actwarm = sp.tile([P, PAIRS], F32, tag="actwarm")
            nc.scalar.activation(actwarm[:], bq_sb[:], AF.Exp, scale=1.0)
            # xT SBUF layout is slice-major so each half-slice load is fully
            # contiguous on both sides; halves split across the two queues.
            # Queue balance tuned to need-by time: each x slice is split in
            # dc-halves across the two queues; pair-1 QK weights and Wo are
            # only needed tens of us in, so they ride at the back.
            xT4 = pp.tile([P, QT_TILES, DCH, QW], BF16)
            HC = DCH // 2
            nc.sync.dma_start(wk_sb[:, :, 0:P], wk0[:])
            nc.gpsimd.dma_start(wq_sb[:, :, 0:P], wq0[:])
            nc.sync.dma_start(xT4[:, 0, 0:HC], xt[:, 0, 0:HC])
            nc.gpsimd.dma_start(xT4[:, 0, HC:DCH], xt[:, 0, HC:DCH])
            nc.sync.dma_start(xT4[:, 1, 0:HC], xt[:, 1, 0:HC])
            nc.gpsimd.dma_start(xT4[:, 1, HC:DCH], xt[:, 1, HC:DCH])
            nc.gpsimd.dma_start(wv_sb[:], wv[:])
            nc.sync.dma_start(xT4[:, 2, 0:HC], xt[:, 2, 0:HC])
            nc.gpsimd.dma_start(xT4[:, 2, HC:DCH], xt[:, 2, HC:DCH])
            nc.sync.dma_start(xT4[:, 3, 0:HC], xt[:, 3, 0:HC])
            nc.gpsimd.dma_start(xT4[:, 3, HC:DCH], xt[:, 3, HC:DCH])
            nc.sync.dma_start(wq_sb[:, :, P:256], wq1[:])
            nc.sync.dma_start(wk_sb[:, :, P:256], wk1[:])
            nc.sync.dma_start(wo_sb[:], wo[:])

            QT = pp.tile([P, PAIRS, S], BF16)
            KT = pp.tile([P, PAIRS, S], BF16)
            VE = pp.tile([P, SC_CHUNKS, HPC * P], BF16)
            ctxT = pp.tile([P, PAIRS, S], BF16)

            # mask columns of V_ext sit FIRST per head (so the softmax denom
            # lands on PSUM partition 0, readable by the fast-reciprocal
            # custom op directly); V columns follow at 1..DH
            ve4 = VE[:].rearrange("p st (h c) -> p st h c", h=HPC)
            nc.vector.tensor_copy(
                ve4[:, :, :, 0:1],
                maskp[:, :, None, None].to_broadcast([P, SC_CHUNKS, HPC, 1]),
            )

            def v_proj(st):
                j, off = st // 4, (st % 4) * P
                pv = ps_w.tile([P, QW], F32, tag="w")
                for dc in range(DCH):
                    nc.tensor.matmul(
                        pv[:, :256],
                        xT4[:, j, dc, off : off + P],
                        wv_sb[:, dc, :],
                        start=(dc == 0),
                        stop=(dc == DCH - 1),
                    )
                nc.vector.tensor_scalar_mul(
                    ve4[:, st, :, DH : 2 * DH],
                    pv[:, :256].rearrange("p (h c) -> p h c", h=HPC),
                    maskp[:, st : st + 1],
                )

            def qk_proj(pr, qt):
                sl = slice(qt * QW, (qt + 1) * QW)
                for dst, w_sb, b_sb in ((QT, wq_sb, bq_sb), (KT, wk_sb, bk_sb)):
                    pq = ps_w.tile([P, QW], F32, tag="w")
                    for dc in range(DCH):
                        nc.tensor.matmul(
                            pq[:],
                            w_sb[:, dc, pr * P : (pr + 1) * P],
                            xT4[:, qt, dc, :],
                            start=(dc == 0),
                            stop=(dc == DCH - 1),
                        )
                    nc.vector.tensor_scalar_add(
                        dst[:, pr, sl], pq[:], b_sb[:, pr : pr + 1]
                    )

            att_state = {}

            def _emit_ctx(pr, qt, cps, et, kc):
                for hh in range(2):
                    h = 2 * pr + hh
                    nc.tensor.matmul(
                        cps[hh][:, :],
                        VE[:, kc, h * P : (h + 1) * P],
                        et[:, hh * QW : (hh + 1) * QW],
                        start=(kc == 0),
                        stop=(kc == SC_CHUNKS - 1),
                    )

            def attn_chunks(pr, qt, kcs):
                qsl = slice(qt * QW, (qt + 1) * QW)
                if (pr, qt) not in att_state:
                    att_state[(pr, qt)] = [
                        [
                            ps_ctx.tile([P, QW], F32, tag="ctx", name=f"ctx{hh}")
                            for hh in range(2)
                        ],
                        [],  # pending (et, kc) whose ctx is not yet emitted
                    ]
                st_ = att_state[(pr, qt)]
                cps, pending = st_
                for kc in kcs:
                    sc = ps_sc.tile([P, 2 * QW], F32, tag="sc")
                    for hh in range(2):
                        nc.tensor.matmul(
                            sc[:, hh * QW : (hh + 1) * QW],
                            KT[hh * DH : (hh + 1) * DH, pr, kc * P : (kc + 1) * P],
                            QT[hh * DH : (hh + 1) * DH, pr, qsl],
                            start=True,
                            stop=True,
                            tile_position=(hh * DH, 0),
                        )
                    et = ep.tile([P, 2 * QW], BF16, tag="et")
                    nc.scalar.activation(et[:], sc[:], AF.Exp, scale=float(SCALE))
                    # ctx lags CTX_LAG chunks so upcoming scores aren't stuck
                    # behind ctx's wait-for-exp in the PE's in-order stream
                    # (also covers the previous qtile's normalization latency
                    # at qt boundaries before ctx's PSUM slot is reusable)
                    pending.append((et, kc))
                    if len(pending) > CTX_LAG:
                        _emit_ctx(pr, qt, cps, *pending.pop(0))

            def attn_norm_fine(pr, qt):
                # last-tile tail: flush ctx, then normalize per 128-query
                # slice and issue each out_proj as soon as its slice is ready
                qsl0 = qt * QW
                cps, pending = att_state.pop((pr, qt))
                for p_ in pending:
                    _emit_ctx(pr, qt, cps, *p_)
                for i in range(4):
                    csl = slice(i * P, (i + 1) * P)
                    for hh in range(2):
                        denr = sp.tile([1, P], F32, tag="denrf", name=f"dnf{hh}")
                        nc.vector.reciprocal_approx_fast(
                            denr[:], cps[hh][0:1, csl]
                        )
                        recB = sp.tile([DH, P], F32, tag="recBf", name=f"rbf{hh}")
                        nc.gpsimd.partition_broadcast(recB[:], denr[:])
                        nc.vector.tensor_mul(
                            ctxT[hh * DH : (hh + 1) * DH, pr, qsl0 + i * P : qsl0 + (i + 1) * P],
                            cps[hh][DH : 2 * DH, csl],
                            recB[:],
                        )
                    out_proj(4 * qt + i)

            def attn_norm_fine(pr, qt):
                # last-tile tail: flush ctx, then normalize per 128-query
                # slice and issue each out_proj as soon as its slice is ready
                qsl0 = qt * QW
                cps, pending = att_state.pop((pr, qt))
                for p_ in pending:
                    _emit_ctx(pr, qt, cps, *p_)
                for i in range(4):
                    csl = slice(i * P, (i + 1) * P)
                    for hh in range(2):
                        denr = sp.tile([1, P], F32, tag="denrf", name=f"dnf{hh}")
                        nc.vector.reciprocal_approx_fast(
                            denr[:], cps[hh][0:1, csl]
                        )
                        recB = sp.tile([DH, P], F32, tag="recBf", name=f"rbf{hh}")
                        nc.gpsimd.partition_broadcast(recB[:], denr[:])
                        nc.vector.tensor_mul(
                            ctxT[hh * DH : (hh + 1) * DH, pr, qsl0 + i * P : qsl0 + (i + 1) * P],
                            cps[hh][DH : 2 * DH, csl],
                            recB[:],
                        )
                    out_proj(4 * qt + i)

            def attn_norm(pr, qt):
                qsl = slice(qt * QW, (qt + 1) * QW)
                cps, pending = att_state.pop((pr, qt))
                for p_ in pending:
                    _emit_ctx(pr, qt, cps, *p_)
                # normalize: recip(den@partition0) -> broadcast -> ctx*rec
                for hh in range(2):
                    denr = sp.tile([1, QW], F32, tag="denr", name=f"denr{hh}")
                    nc.vector.reciprocal_approx_fast(denr[:], cps[hh][0:1, :])
                    recB = sp.tile([DH, QW], F32, tag="recB", name=f"recB{hh}")
                    nc.gpsimd.partition_broadcast(recB[:], denr[:])
                    nc.vector.tensor_mul(
                        ctxT[hh * DH : (hh + 1) * DH, pr, qsl],
                        cps[hh][DH : 2 * DH, :],
                        recB[:],
                    )

            def out_proj(st):
                ob = op_.tile([P, D], BF16, tag="ob")
                for nt in range(2):
                    po = ps_w.tile([P, QW], F32, tag="w")
                    for pr in range(PAIRS):
                        nc.tensor.matmul(
                            po[:],
                            ctxT[:, pr, st * P : (st + 1) * P],
                            wo_sb[:, pr, nt * QW : (nt + 1) * QW],
                            start=(pr == 0),
                            stop=(pr == PAIRS - 1),
                        )
                    nc.vector.tensor_copy(ob[:, nt * QW : (nt + 1) * QW], po[:])
                # mid-phase stores stay off the gpsimd queue (it serves the
                # normalization broadcasts); only the final burst splits
                eng = nc.gpsimd if st >= 12 and st % 2 == 1 else nc.sync
                eng.dma_start(out[st * P : (st + 1) * P, :], ob[:])

            def attention(pr, qt):
                attn_chunks(pr, qt, range(SC_CHUNKS))
                attn_norm(pr, qt)

            # ---- emission order (sets scheduling priority) ----
            # Chase: attention(0,{0,1}) kc-chunks interleave with the pair-0
            # QK / V projection chase so exp starts early and has 2 qtiles of
            # runway per x slice; later projections (pair 1, out) are emitted
            # one step late so the next scores outrank them and ACT never
            # starves.
            # chase: v_proj(st) is emitted just before the chunk that first
            # needs VE[st] (ctx lags CTX_LAG chunks), keeping the PE stream
            # ahead of ACT as tightly as the serial deps allow
            qk_proj(0, 0)
            attn_chunks(0, 0, range(0, 3))
            v_proj(0)
            attn_chunks(0, 0, range(3, 4))
            for j in range(1, QT_TILES):
                qk_proj(0, j)
                for kc in range(4 * j, 4 * j + 4):
                    v_proj(kc - CTX_LAG)
                    attn_chunks(0, 0, range(kc, kc + 1))
            for st in range(16 - CTX_LAG, 16):
                v_proj(st)
            attn_norm(0, 0)
            attention(0, 1)
            qk_proj(1, 0)
            attention(0, 2)
            qk_proj(1, 1)
            attention(0, 3)
            qk_proj(1, 2)
            attn_chunks(1, 0, range(0, 8))
            qk_proj(1, 3)
            attn_chunks(1, 0, range(8, 16))
            attn_norm(1, 0)
            attention(1, 1)
            for st in range(0, 4):
                out_proj(st)
            attn_chunks(1, 2, range(0, 8))
            out_proj(4)
            out_proj(5)
            attn_chunks(1, 2, range(8, 16))
            attn_norm(1, 2)
            out_proj(6)
            out_proj(7)
            attn_chunks(1, 3, range(0, 6))
            out_proj(8)
            out_proj(9)
            attn_chunks(1, 3, range(6, 12))
            out_proj(10)
            out_proj(11)
            attn_chunks(1, 3, range(12, 16))
            attn_norm_fine(1, 3)

    nc.finalize()
    return nc


def ts(i, w):
    return slice(i * w, (i + 1) * w)


def _sb_w(w):
    """[D, n] weight slice -> SBUF layout [P, DCH, n]."""
    return np.ascontiguousarray(np.asarray(w).reshape(DCH, P, -1).transpose(1, 0, 2))


def shard_inputs(x, Wq, bq, Wk, bk, Wv, bv, Wo, bo, mask):
    """Full inputs -> list of 8 per-core input maps, pre-tiled to SBUF
    layouts (pure host-side layout prep; no kernel math moves to host)."""
    maskf = (~np.asarray(mask)).astype(np.float32)  # 1.0 = keep
    bf16 = ml_dtypes.bfloat16
    x = np.asarray(x, dtype=np.float32)
    # xt[p, j, c, s] = x[j*QW+s, c*P+p]
    xts = [
        np.ascontiguousarray(
            x[b].T.reshape(DCH, P, QT_TILES, QW).transpose(1, 2, 0, 3)
        ).astype(bf16)
        for b in range(2)
    ]
    Wqh = np.asarray(Wq, np.float32).astype(bf16)
    Wkh = np.asarray(Wk, np.float32).astype(bf16)
    Wvh = np.asarray(Wv, np.float32).astype(bf16)
    Woh = np.asarray(Wo, np.float32).astype(bf16)
    mask_t = [
        np.ascontiguousarray(maskf[b].reshape(SC_CHUNKS, P).T) for b in range(2)
    ]
    ins = []
    for c in range(N_CORES):
        b, g = divmod(c, 4)
        cs = slice(g * 256, (g + 1) * 256)
        wq_t = _sb_w(Wqh[:, cs])
        wk_t = _sb_w(Wkh[:, cs])
        ins.append(
            {
                "xt": xts[b],
                "wq0": np.ascontiguousarray(wq_t[:, :, 0:P]),
                "wq1": np.ascontiguousarray(wq_t[:, :, P:256]),
                "wk0": np.ascontiguousarray(wk_t[:, :, 0:P]),
                "wk1": np.ascontiguousarray(wk_t[:, :, P:256]),
                "wv": _sb_w(Wvh[:, cs]),
                "wo": np.ascontiguousarray(
                    Woh[cs, :].reshape(PAIRS, P, D).transpose(1, 0, 2)
                ),
                "bq": np.ascontiguousarray(
                    np.asarray(bq, np.float32)[cs].reshape(PAIRS, P).T
                ),
                "bk": np.ascontiguousarray(
                    np.asarray(bk, np.float32)[cs].reshape(PAIRS, P).T
                ),
                "maskf": mask_t[b],
            }
        )
    return ins


def gather_outputs(results, bv, Wo, bo):
    """8 per-core partial outputs -> full (2, S, D) fp32 output."""
    bo_eff = np.asarray(bo, dtype=np.float32) + np.asarray(
        bv, dtype=np.float32
    ) @ np.asarray(Wo, dtype=np.float32)
    outs = []
    for b in range(2):
        acc = results[4 * b]["out"].astype(np.float32).copy()
        for g in range(1, 4):
            acc += results[4 * b + g]["out"]
        outs.append(acc + bo_eff)
    return np.stack(outs, axis=0)


_NC_CACHE = []


def _get_nc():
    if not _NC_CACHE:
        _NC_CACHE.append(build())
    return _NC_CACHE[0]


def run_sharded(inputs, trace=False, tmpdir=None):
    """Shard, run on cores 0-7, gather. Returns (output, BassKernelResults)."""
    nc = _get_nc()
    ins = shard_inputs(**inputs)
    res = run_bass_kernel_spmd(
        nc, ins, core_ids=list(range(N_CORES)), trace=trace, tmpdir=tmpdir
    )
    full = gather_outputs(res.results, inputs["bv"], inputs["Wo"], inputs["bo"])
    return full, res


def kernel(**inputs) -> np.ndarray:
    full, _ = run_sharded(inputs, trace=False)
    return full


# revision 49
# speedup vs baseline: 1.0314x; 1.0314x over previous
"""Multi-head attention (B=2, S=2048, D=1024, H=16, Dh=64) on 8 Trainium2
NeuronCores via Bass/Tile.

Sharding: data-parallel over the 2 batches x tensor-parallel over head
groups (16 heads -> 4 groups of 4). Core c = 4*b + g handles batch b and
heads 4g..4g+3 with the matching column/row slices of Wq/Wk/Wv/Wo. Each
core returns its partial output projection; the host sums the 4 partials
per batch and adds bo' (bo' = bo + bv @ Wo, so the V bias never has to be
applied on device: softmax weights sum to 1, so ctx = sum_k p_k (xWv)_k
+ bv, and the bv term commutes through Wo).

Host passes x pre-transposed (xT, [D, S]) so the kernel needs no on-chip
transposes. Per-core kernel (4 heads = 2 pairs of 64-dim heads stacked on
the 128-partition dim), bf16 matmul datapath with fp32 PSUM accumulation:
  QT   = Wq_g^T x^T + bq_g              [128 (2 heads x 64), 2 pairs, S]
  KT   = Wk_g^T x^T + bk_g              (same layout)
  V_ext= [(x Wv_g) * maskf | maskf]     [s, chunk, 4*(64+1)] bf16
  per pair, per q-tile (512 queries), per key chunk (128 keys):
    scT [128k, 2x512q] = KT_chunk^T @ QT_tile   (2 heads row-packed in PE)
    eT  = exp(SCALE * scT)                      (one ACT op per pair, bf16)
    ctx_h[65, 512] += V_ext_chunk^T @ eT_h      (row 64 = softmax denom)
  normalize: recip(den) -> broadcast (GPSIMD) -> ctxT = ctx*rec
  out_partial = ctxT^T @ Wo_g           (PSUM accum over the 2 pairs)

The masked-softmax trick: exp is taken over unmasked scores (safe: |score*
SCALE| < ~3 here), and the 0/1 key mask is folded into V_ext (zeroed V rows
and the mask column), so masked keys contribute 0 to both the numerator and
the denominator -- no -inf arithmetic on device.

Emission order software-pipelines the ACT (exp) engine: QK projections for
pair 0 are chased by attention(0, qt) so the first exp issues ~6us in, and
V projections / pair-1 projections / output projections fill PE slack while
ACT streams exp chunks back-to-back (ACT is the bottleneck engine at ~147us
busy; total PE work is ~137us).
"""

import ml_dtypes
import numpy as np

import concourse.bacc as bacc
import concourse.mybir as mybir
import concourse.tile as tile
from concourse.bass_utils import run_bass_kernel_spmd

F32 = mybir.dt.float32
BF16 = mybir.dt.bfloat16
AF = mybir.ActivationFunctionType

S = 2048
D = 1024
HPC = 4                  # heads per core
DH = 64
PAIRS = 2                # head pairs per core
P = 128
SC_CHUNKS = S // P       # 16 key chunks
QT_TILES = 4             # q tiles of 512
QW = 512                 # q tile width
ST_TILES = S // P        # 16 s tiles
DCH = D // P             # 8 D chunks
SCALE = 1.0 / np.sqrt(DH)
CTX_LAG = 3

N_CORES = 8


def build():
    nc = bacc.Bacc(None, target_bir_lowering=False, num_swdge_queues=4)

    # All inputs arrive host-pre-tiled in their exact SBUF layouts (bf16 where
    # the datapath is bf16) so every load DMA reads big contiguous lines.
    xt = nc.dram_tensor("xt", [P, QT_TILES, DCH, QW], BF16, kind="ExternalInput")
    wq0 = nc.dram_tensor("wq0", [P, DCH, P], BF16, kind="ExternalInput")
    wq1 = nc.dram_tensor("wq1", [P, DCH, P], BF16, kind="ExternalInput")
    wk0 = nc.dram_tensor("wk0", [P, DCH, P], BF16, kind="ExternalInput")
    wk1 = nc.dram_tensor("wk1", [P, DCH, P], BF16, kind="ExternalInput")
    wv = nc.dram_tensor("wv", [P, DCH, 256], BF16, kind="ExternalInput")
    wo = nc.dram_tensor("wo", [P, PAIRS, D], BF16, kind="ExternalInput")
    bq = nc.dram_tensor("bq", [P, PAIRS], F32, kind="ExternalInput")
    bk = nc.dram_tensor("bk", [P, PAIRS], F32, kind="ExternalInput")
    maskf = nc.dram_tensor("maskf", [P, SC_CHUNKS], F32, kind="ExternalInput")
    out = nc.dram_tensor("out", [S, D], BF16, kind="ExternalOutput")

    with tile.TileContext(nc) as tc:
        with (
            tc.tile_pool(name="persist", bufs=1) as pp,
            tc.tile_pool(name="expp", bufs=8) as ep,
            tc.tile_pool(name="ostage", bufs=2) as op_,
            tc.tile_pool(name="smalls", bufs=3) as sp,
            tc.tile_pool(name="ps_sc", bufs=2, space="PSUM") as ps_sc,
            tc.tile_pool(name="ps_ctx", bufs=2, space="PSUM") as ps_ctx,
            tc.tile_pool(name="ps_w", bufs=2, space="PSUM") as ps_w,
        ):
            # ---- constants / persistent tensors ----
            # weights first: qk_proj(0, 0) needs wq/wk asap
            wq_sb = pp.tile([P, DCH, 256], BF16)
            wk_sb = pp.tile([P, DCH, 256], BF16)
            wv_sb = pp.tile([P, DCH, 256], BF16)
            wo_sb = pp.tile([P, PAIRS, D], BF16)
            # casting DMAs (fp32 DRAM -> bf16 SBUF) must go via gpsimd/SWDGE
            # x slices stream on the HWDGE (sync) queue, weights on the SWDGE
            # (gpsimd) queues, so the qk chase and V projections are never
            # serialized behind each other's loads.
            bq_sb = pp.tile([P, PAIRS], F32)
            bk_sb = pp.tile([P, PAIRS], F32)
            maskp = pp.tile([P, SC_CHUNKS], F32)
            nc.sync.dma_start(maskp[:], maskf[:])
            nc.sync.dma_start(bq_sb[:], bq[:])
            nc.sync.dma_start(bk_sb[:], bk[:])
            # warm the ACT exp table while DMAs stream (table load ~2.7us)
            actwarm = sp.tile([P, PAIRS], F32, tag="actwarm")
            nc.scalar.activation(actwarm[:], bq_sb[:], AF.Exp, scale=1.0)
            # warm the ACT exp table while DMAs stream (table load ~2.7us)
            actwarm = sp.tile([P, PAIRS], F32, tag="actwarm")
            nc.scalar.activation(actwarm[:], bq_sb[:], AF.Exp, scale=1.0)
            # xT SBUF layout is slice-major so each half-slice load is fully
            # contiguous on both sides; halves split across the two queues.
            # Queue balance tuned to need-by time: each x slice is split in
            # dc-halves across the two queues; pair-1 QK weights and Wo are
            # only needed tens of us in, so they ride at the back.
            xT4 = pp.tile([P, QT_TILES, DCH, QW], BF16)
            HC = DCH // 2
            nc.sync.dma_start(xT4[:, 0, 0:HC], xt[:, 0, 0:HC])
            nc.gpsimd.dma_start(wq_sb[:, :, 0:P], wq0[:])
            nc.gpsimd.dma_start(wk_sb[:, :, 0:P], wk0[:])
            nc.gpsimd.dma_start(xT4[:, 0, HC:DCH], xt[:, 0, HC:DCH])
            nc.sync.dma_start(xT4[:, 1, 0:HC], xt[:, 1, 0:HC])
            nc.gpsimd.dma_start(xT4[:, 1, HC:DCH], xt[:, 1, HC:DCH])
            nc.gpsimd.dma_start(wv_sb[:], wv[:])
            nc.sync.dma_start(xT4[:, 2, 0:HC], xt[:, 2, 0:HC])
            nc.gpsimd.dma_start(xT4[:, 2, HC:DCH], xt[:, 2, HC:DCH])
            nc.sync.dma_start(xT4[:, 3, 0:HC], xt[:, 3, 0:HC])
            nc.gpsimd.dma_start(xT4[:, 3, HC:DCH], xt[:, 3, HC:DCH])
            nc.sync.dma_start(wq_sb[:, :, P:256], wq1[:])
            nc.sync.dma_start(wk_sb[:, :, P:256], wk1[:])
            nc.sync.dma_start(wo_sb[:], wo[:])

            QT = pp.tile([P, PAIRS, S], BF16)
            KT = pp.tile([P, PAIRS, S], BF16)
            VE = pp.tile([P, SC_CHUNKS, HPC * P], BF16)
            ctxT = pp.tile([P, PAIRS, S], BF16)

            # mask columns of V_ext sit FIRST per head (so the softmax denom
            # lands on PSUM partition 0, readable by the fast-reciprocal
            # custom op directly); V columns follow at 1..DH
            ve4 = VE[:].rearrange("p st (h c) -> p st h c", h=HPC)
            nc.vector.tensor_copy(
                ve4[:, :, :, 0:1],
                maskp[:, :, None, None].to_broadcast([P, SC_CHUNKS, HPC, 1]),
            )

            def v_proj(st):
                j, off = st // 4, (st % 4) * P
                pv = ps_w.tile([P, QW], F32, tag="w")
                for dc in range(DCH):
                    nc.tensor.matmul(
                        pv[:, :256],
                        xT4[:, j, dc, off : off + P],
                        wv_sb[:, dc, :],
                        start=(dc == 0),
                        stop=(dc == DCH - 1),
                    )
                nc.vector.tensor_scalar_mul(
                    ve4[:, st, :, DH : 2 * DH],
                    pv[:, :256].rearrange("p (h c) -> p h c", h=HPC),
                    maskp[:, st : st + 1],
                )

            def qk_proj(pr, qt):
                sl = slice(qt * QW, (qt + 1) * QW)
                for dst, w_sb, b_sb in ((QT, wq_sb, bq_sb), (KT, wk_sb, bk_sb)):
                    pq = ps_w.tile([P, QW], F32, tag="w")
                    for dc in range(DCH):
                        nc.tensor.matmul(
                            pq[:],
                            w_sb[:, dc, pr * P : (pr + 1) * P],
                            xT4[:, qt, dc, :],
                            start=(dc == 0),
                            stop=(dc == DCH - 1),
                        )
                    nc.vector.tensor_scalar_add(
                        dst[:, pr, sl], pq[:], b_sb[:, pr : pr + 1]
                    )

            att_state = {}

            def _emit_ctx(pr, qt, cps, et, kc):
                for hh in range(2):
                    h = 2 * pr + hh
                    nc.tensor.matmul(
                        cps[hh][:, :],
                        VE[:, kc, h * P : (h + 1) * P],
                        et[:, hh * QW : (hh + 1) * QW],
                        start=(kc == 0),
                        stop=(kc == SC_CHUNKS - 1),
                    )

            def attn_chunks(pr, qt, kcs):
                qsl = slice(qt * QW, (qt + 1) * QW)
                if (pr, qt) not in att_state:
                    att_state[(pr, qt)] = [
                        [
                            ps_ctx.tile([P, QW], F32, tag="ctx", name=f"ctx{hh}")
                            for hh in range(2)
                        ],
                        [],  # pending (et, kc) whose ctx is not yet emitted
                    ]
                st_ = att_state[(pr, qt)]
                cps, pending = st_
                for kc in kcs:
                    sc = ps_sc.tile([P, 2 * QW], F32, tag="sc")
                    for hh in range(2):
                        nc.tensor.matmul(
                            sc[:, hh * QW : (hh + 1) * QW],
                            KT[hh * DH : (hh + 1) * DH, pr, kc * P : (kc + 1) * P],
                            QT[hh * DH : (hh + 1) * DH, pr, qsl],
                            start=True,
                            stop=True,
                            tile_position=(hh * DH, 0),
                        )
                    et = ep.tile([P, 2 * QW], BF16, tag="et")
                    nc.scalar.activation(et[:], sc[:], AF.Exp, scale=float(SCALE))
                    # ctx lags CTX_LAG chunks so upcoming scores aren't stuck
                    # behind ctx's wait-for-exp in the PE's in-order stream
                    # (also covers the previous qtile's normalization latency
                    # at qt boundaries before ctx's PSUM slot is reusable)
                    pending.append((et, kc))
                    if len(pending) > CTX_LAG:
                        _emit_ctx(pr, qt, cps, *pending.pop(0))

            def attn_norm_fine(pr, qt):
                # last-tile tail: flush ctx, then normalize per 128-query
                # slice and issue each out_proj as soon as its slice is ready
                qsl0 = qt * QW
                cps, pending = att_state.pop((pr, qt))
                for p_ in pending:
                    _emit_ctx(pr, qt, cps, *p_)
                for i in range(4):
                    csl = slice(i * P, (i + 1) * P)
                    for hh in range(2):
                        denr = sp.tile([1, P], F32, tag="denrf", name=f"dnf{hh}")
                        nc.vector.reciprocal_approx_fast(
                            denr[:], cps[hh][0:1, csl]
                        )
                        recB = sp.tile([DH, P], F32, tag="recBf", name=f"rbf{hh}")
                        nc.gpsimd.partition_broadcast(recB[:], denr[:])
                        nc.vector.tensor_mul(
                            ctxT[hh * DH : (hh + 1) * DH, pr, qsl0 + i * P : qsl0 + (i + 1) * P],
                            cps[hh][DH : 2 * DH, csl],
                            recB[:],
                        )
                    out_proj(4 * qt + i)

            def attn_norm_fine(pr, qt):
                # last-tile tail: flush ctx, then normalize per 128-query
                # slice and issue each out_proj as soon as its slice is ready
                qsl0 = qt * QW
                cps, pending = att_state.pop((pr, qt))
                for p_ in pending:
                    _emit_ctx(pr, qt, cps, *p_)
                for i in range(4):
                    csl = slice(i * P, (i + 1) * P)
                    for hh in range(2):
                        denr = sp.tile([1, P], F32, tag="denrf", name=f"dnf{hh}")
                        nc.vector.reciprocal_approx_fast(
                            denr[:], cps[hh][0:1, csl]
                        )
                        recB = sp.tile([DH, P], F32, tag="recBf", name=f"rbf{hh}")
                        nc.gpsimd.partition_broadcast(recB[:], denr[:])
                        nc.vector.tensor_mul(
                            ctxT[hh * DH : (hh + 1) * DH, pr, qsl0 + i * P : qsl0 + (i + 1) * P],
                            cps[hh][DH : 2 * DH, csl],
                            recB[:],
                        )
                    out_proj(4 * qt + i)

            def attn_norm(pr, qt):
                qsl = slice(qt * QW, (qt + 1) * QW)
                cps, pending = att_state.pop((pr, qt))
                for p_ in pending:
                    _emit_ctx(pr, qt, cps, *p_)
                # normalize: recip(den@partition0) -> broadcast -> ctx*rec
                for hh in range(2):
                    denr = sp.tile([1, QW], F32, tag="denr", name=f"denr{hh}")
                    nc.vector.reciprocal_approx_fast(denr[:], cps[hh][0:1, :])
                    recB = sp.tile([DH, QW], F32, tag="recB", name=f"recB{hh}")
                    nc.gpsimd.partition_broadcast(recB[:], denr[:])
                    nc.vector.tensor_mul(
                        ctxT[hh * DH : (hh + 1) * DH, pr, qsl],
                        cps[hh][DH : 2 * DH, :],
                        recB[:],
                    )

            def out_proj(st):
                ob = op_.tile([P, D], BF16, tag="ob")
                for nt in range(2):
                    po = ps_w.tile([P, QW], F32, tag="w")
                    for pr in range(PAIRS):
                        nc.tensor.matmul(
                            po[:],
                            ctxT[:, pr, st * P : (st + 1) * P],
                            wo_sb[:, pr, nt * QW : (nt + 1) * QW],
                            start=(pr == 0),
                            stop=(pr == PAIRS - 1),
                        )
                    nc.vector.tensor_copy(ob[:, nt * QW : (nt + 1) * QW], po[:])
                # mid-phase stores stay off the gpsimd queue (it serves the
                # normalization broadcasts); only the final burst splits
                eng = nc.gpsimd if st >= 12 and st % 2 == 1 else nc.sync
                eng.dma_start(out[st * P : (st + 1) * P, :], ob[:])

            def attention(pr, qt):
                attn_chunks(pr, qt, range(SC_CHUNKS))
                attn_norm(pr, qt)

            # ---- emission order (sets scheduling priority) ----
            # Chase: attention(0,{0,1}) kc-chunks interleave with the pair-0
            # QK / V projection chase so exp starts early and has 2 qtiles of
            # runway per x slice; later projections (pair 1, out) are emitted
            # one step late so the next scores outrank them and ACT never
            # starves.
            for j in range(QT_TILES):
                qk_proj(0, j)
                attn_chunks(0, 0, range(4 * j, 4 * j + 2))
                for st in range(4 * j, 4 * j + 4):
                    v_proj(st)
                attn_chunks(0, 0, range(4 * j + 2, 4 * j + 4))
            attn_norm(0, 0)
            attention(0, 1)
            qk_proj(1, 0)
            attention(0, 2)
            qk_proj(1, 1)
            attention(0, 3)
            qk_proj(1, 2)
            attn_chunks(1, 0, range(0, 8))
            qk_proj(1, 3)
            attn_chunks(1, 0, range(8, 16))
            attn_norm(1, 0)
            attention(1, 1)
            for st in range(0, 4):
                out_proj(st)
            attn_chunks(1, 2, range(0, 8))
            out_proj(4)
            out_proj(5)
            attn_chunks(1, 2, range(8, 16))
            attn_norm(1, 2)
            out_proj(6)
            out_proj(7)
            attn_chunks(1, 3, range(0, 6))
            out_proj(8)
            out_proj(9)
            attn_chunks(1, 3, range(6, 12))
            out_proj(10)
            out_proj(11)
            attn_chunks(1, 3, range(12, 16))
            attn_norm_fine(1, 3)

    nc.finalize()
    return nc


def ts(i, w):
    return slice(i * w, (i + 1) * w)


def _sb_w(w):
    """[D, n] weight slice -> SBUF layout [P, DCH, n]."""
    return np.ascontiguousarray(np.asarray(w).reshape(DCH, P, -1).transpose(1, 0, 2))


def shard_inputs(x, Wq, bq, Wk, bk, Wv, bv, Wo, bo, mask):
    """Full inputs -> list of 8 per-core input maps, pre-tiled to SBUF
    layouts (pure host-side layout prep; no kernel math moves to host)."""
    maskf = (~np.asarray(mask)).astype(np.float32)  # 1.0 = keep
    bf16 = ml_dtypes.bfloat16
    x = np.asarray(x, dtype=np.float32)
    # xt[p, j, c, s] = x[j*QW+s, c*P+p]
    xts = [
        np.ascontiguousarray(
            x[b].T.reshape(DCH, P, QT_TILES, QW).transpose(1, 2, 0, 3)
        ).astype(bf16)
        for b in range(2)
    ]
    Wqh = np.asarray(Wq, np.float32).astype(bf16)
    Wkh = np.asarray(Wk, np.float32).astype(bf16)
    Wvh = np.asarray(Wv, np.float32).astype(bf16)
    Woh = np.asarray(Wo, np.float32).astype(bf16)
    mask_t = [
        np.ascontiguousarray(maskf[b].reshape(SC_CHUNKS, P).T) for b in range(2)
    ]
    ins = []
    for c in range(N_CORES):
        b, g = divmod(c, 4)
        cs = slice(g * 256, (g + 1) * 256)
        wq_t = _sb_w(Wqh[:, cs])
        wk_t = _sb_w(Wkh[:, cs])
        ins.append(
            {
                "xt": xts[b],
                "wq0": np.ascontiguousarray(wq_t[:, :, 0:P]),
                "wq1": np.ascontiguousarray(wq_t[:, :, P:256]),
                "wk0": np.ascontiguousarray(wk_t[:, :, 0:P]),
                "wk1": np.ascontiguousarray(wk_t[:, :, P:256]),
                "wv": _sb_w(Wvh[:, cs]),
                "wo": np.ascontiguousarray(
                    Woh[cs, :].reshape(PAIRS, P, D).transpose(1, 0, 2)
                ),
                "bq": np.ascontiguousarray(
                    np.asarray(bq, np.float32)[cs].reshape(PAIRS, P).T
                ),
                "bk": np.ascontiguousarray(
                    np.asarray(bk, np.float32)[cs].reshape(PAIRS, P).T
                ),
                "maskf": mask_t[b],
            }
        )
    return ins


def gather_outputs(results, bv, Wo, bo):
    """8 per-core partial outputs -> full (2, S, D) fp32 output."""
    bo_eff = np.asarray(bo, dtype=np.float32) + np.asarray(
        bv, dtype=np.float32
    ) @ np.asarray(Wo, dtype=np.float32)
    outs = []
    for b in range(2):
        acc = results[4 * b]["out"].astype(np.float32).copy()
        for g in range(1, 4):
            acc += results[4 * b + g]["out"]
        outs.append(acc + bo_eff)
    return np.stack(outs, axis=0)


_NC_CACHE = []


def _get_nc():
    if not _NC_CACHE:
        _NC_CACHE.append(build())
    return _NC_CACHE[0]


def run_sharded(inputs, trace=False, tmpdir=None):
    """Shard, run on cores 0-7, gather. Returns (output, BassKernelResults)."""
    nc = _get_nc()
    ins = shard_inputs(**inputs)
    res = run_bass_kernel_spmd(
        nc, ins, core_ids=list(range(N_CORES)), trace=trace, tmpdir=tmpdir
    )
    full = gather_outputs(res.results, inputs["bv"], inputs["Wo"], inputs["bo"])
    return full, res


def kernel(**inputs) -> np.ndarray:
    full, _ = run_sharded(inputs, trace=False)
    return full


# revision 50
# speedup vs baseline: 1.0389x; 1.0073x over previous
"""Multi-head attention (B=2, S=2048, D=1024, H=16, Dh=64) on 8 Trainium2
NeuronCores via Bass/Tile.

Sharding: data-parallel over the 2 batches x tensor-parallel over head
groups (16 heads -> 4 groups of 4). Core c = 4*b + g handles batch b and
heads 4g..4g+3 with the matching column/row slices of Wq/Wk/Wv/Wo. Each
core returns its partial output projection; the host sums the 4 partials
per batch and adds bo' (bo' = bo + bv @ Wo, so the V bias never has to be
applied on device: softmax weights sum to 1, so ctx = sum_k p_k (xWv)_k
+ bv, and the bv term commutes through Wo).

Host passes x pre-transposed (xT, [D, S]) so the kernel needs no on-chip
transposes. Per-core kernel (4 heads = 2 pairs of 64-dim heads stacked on
the 128-partition dim), bf16 matmul datapath with fp32 PSUM accumulation:
  QT   = Wq_g^T x^T + bq_g              [128 (2 heads x 64), 2 pairs, S]
  KT   = Wk_g^T x^T + bk_g              (same layout)
  V_ext= [(x Wv_g) * maskf | maskf]     [s, chunk, 4*(64+1)] bf16
  per pair, per q-tile (512 queries), per key chunk (128 keys):
    scT [128k, 2x512q] = KT_chunk^T @ QT_tile   (2 heads row-packed in PE)
    eT  = exp(SCALE * scT)                      (one ACT op per pair, bf16)
    ctx_h[65, 512] += V_ext_chunk^T @ eT_h      (row 64 = softmax denom)
  normalize: recip(den) -> broadcast (GPSIMD) -> ctxT = ctx*rec
  out_partial = ctxT^T @ Wo_g           (PSUM accum over the 2 pairs)

The masked-softmax trick: exp is taken over unmasked scores (safe: |score*
SCALE| < ~3 here), and the 0/1 key mask is folded into V_ext (zeroed V rows
and the mask column), so masked keys contribute 0 to both the numerator and
the denominator -- no -inf arithmetic on device.

Emission order software-pipelines the ACT (exp) engine: QK projections for
pair 0 are chased by attention(0, qt) so the first exp issues ~6us in, and
V projections / pair-1 projections / output projections fill PE slack while
ACT streams exp chunks back-to-back (ACT is the bottleneck engine at ~147us
busy; total PE work is ~137us).
"""

import ml_dtypes
import numpy as np

import concourse.bacc as bacc
import concourse.mybir as mybir
import concourse.tile as tile
from concourse.bass_utils import run_bass_kernel_spmd

F32 = mybir.dt.float32
BF16 = mybir.dt.bfloat16
AF = mybir.ActivationFunctionType

S = 2048
D = 1024
HPC = 4                  # heads per core
DH = 64
PAIRS = 2                # head pairs per core
P = 128
SC_CHUNKS = S // P       # 16 key chunks
QT_TILES = 4             # q tiles of 512
QW = 512                 # q tile width
ST_TILES = S // P        # 16 s tiles
DCH = D // P             # 8 D chunks
SCALE = 1.0 / np.sqrt(DH)
CTX_LAG = 3

N_CORES = 8


def build():
    nc = bacc.Bacc(None, target_bir_lowering=False, num_swdge_queues=4)

    # All inputs arrive host-pre-tiled in their exact SBUF layouts (bf16 where
    # the datapath is bf16) so every load DMA reads big contiguous lines.
    xt = nc.dram_tensor("xt", [P, QT_TILES, DCH, QW], BF16, kind="ExternalInput")
    wq0 = nc.dram_tensor("wq0", [P, DCH, P], BF16, kind="ExternalInput")
    wq1 = nc.dram_tensor("wq1", [P, DCH, P], BF16, kind="ExternalInput")
    wk0 = nc.dram_tensor("wk0", [P, DCH, P], BF16, kind="ExternalInput")
    wk1 = nc.dram_tensor("wk1", [P, DCH, P], BF16, kind="ExternalInput")
    wv = nc.dram_tensor("wv", [P, DCH, 256], BF16, kind="ExternalInput")
    wo = nc.dram_tensor("wo", [P, PAIRS, D], BF16, kind="ExternalInput")
    bq = nc.dram_tensor("bq", [P, PAIRS], F32, kind="ExternalInput")
    bk = nc.dram_tensor("bk", [P, PAIRS], F32, kind="ExternalInput")
    maskf = nc.dram_tensor("maskf", [P, SC_CHUNKS], F32, kind="ExternalInput")
    out = nc.dram_tensor("out", [S, D], BF16, kind="ExternalOutput")

    with tile.TileContext(nc) as tc:
        with (
            tc.tile_pool(name="persist", bufs=1) as pp,
            tc.tile_pool(name="expp", bufs=8) as ep,
            tc.tile_pool(name="ostage", bufs=2) as op_,
            tc.tile_pool(name="smalls", bufs=3) as sp,
            tc.tile_pool(name="ps_sc", bufs=2, space="PSUM") as ps_sc,
            tc.tile_pool(name="ps_ctx", bufs=2, space="PSUM") as ps_ctx,
            tc.tile_pool(name="ps_w", bufs=2, space="PSUM") as ps_w,
        ):
            # ---- constants / persistent tensors ----
            # weights first: qk_proj(0, 0) needs wq/wk asap
            wq_sb = pp.tile([P, DCH, 256], BF16)
            wk_sb = pp.tile([P, DCH, 256], BF16)
            wv_sb = pp.tile([P, DCH, 256], BF16)
            wo_sb = pp.tile([P, PAIRS, D], BF16)
            # casting DMAs (fp32 DRAM -> bf16 SBUF) must go via gpsimd/SWDGE
            # x slices stream on the HWDGE (sync) queue, weights on the SWDGE
            # (gpsimd) queues, so the qk chase and V projections are never
            # serialized behind each other's loads.
            bq_sb = pp.tile([P, PAIRS], F32)
            bk_sb = pp.tile([P, PAIRS], F32)
            maskp = pp.tile([P, SC_CHUNKS], F32)
            nc.sync.dma_start(maskp[:], maskf[:])
            nc.sync.dma_start(bq_sb[:], bq[:])
            nc.sync.dma_start(bk_sb[:], bk[:])
            # warm the ACT exp table while DMAs stream (table load ~2.7us)
            actwarm = sp.tile([P, PAIRS], F32, tag="actwarm")
            nc.scalar.activation(actwarm[:], bq_sb[:], AF.Exp, scale=1.0)
            # warm the ACT exp table while DMAs stream (table load ~2.7us)
            actwarm = sp.tile([P, PAIRS], F32, tag="actwarm")
            nc.scalar.activation(actwarm[:], bq_sb[:], AF.Exp, scale=1.0)
            # xT SBUF layout is slice-major so each half-slice load is fully
            # contiguous on both sides; halves split across the two queues.
            # Queue balance tuned to need-by time: each x slice is split in
            # dc-halves across the two queues; pair-1 QK weights and Wo are
            # only needed tens of us in, so they ride at the back.
            xT4 = pp.tile([P, QT_TILES, DCH, QW], BF16)
            HC = DCH // 2
            nc.sync.dma_start(xT4[:, 0, 0:HC], xt[:, 0, 0:HC])
            nc.gpsimd.dma_start(wq_sb[:, :, 0:P], wq0[:])
            nc.gpsimd.dma_start(wk_sb[:, :, 0:P], wk0[:])
            nc.gpsimd.dma_start(xT4[:, 0, HC:DCH], xt[:, 0, HC:DCH])
            nc.sync.dma_start(xT4[:, 1, 0:HC], xt[:, 1, 0:HC])
            nc.gpsimd.dma_start(xT4[:, 1, HC:DCH], xt[:, 1, HC:DCH])
            nc.gpsimd.dma_start(wv_sb[:], wv[:])
            nc.sync.dma_start(xT4[:, 2, 0:HC], xt[:, 2, 0:HC])
            nc.gpsimd.dma_start(xT4[:, 2, HC:DCH], xt[:, 2, HC:DCH])
            nc.sync.dma_start(xT4[:, 3, 0:HC], xt[:, 3, 0:HC])
            nc.gpsimd.dma_start(xT4[:, 3, HC:DCH], xt[:, 3, HC:DCH])
            nc.sync.dma_start(wq_sb[:, :, P:256], wq1[:])
            nc.sync.dma_start(wk_sb[:, :, P:256], wk1[:])
            nc.sync.dma_start(wo_sb[:], wo[:])

            QT = pp.tile([P, PAIRS, S], BF16)
            KT = pp.tile([P, PAIRS, S], BF16)
            VE = pp.tile([P, SC_CHUNKS, HPC * P], BF16)
            ctxT = pp.tile([P, PAIRS, S], BF16)

            # mask columns of V_ext sit FIRST per head (so the softmax denom
            # lands on PSUM partition 0, readable by the fast-reciprocal
            # custom op directly); V columns follow at 1..DH
            ve4 = VE[:].rearrange("p st (h c) -> p st h c", h=HPC)
            nc.vector.tensor_copy(
                ve4[:, :, :, 0:1],
                maskp[:, :, None, None].to_broadcast([P, SC_CHUNKS, HPC, 1]),
            )

            def v_proj(st):
                j, off = st // 4, (st % 4) * P
                pv = ps_w.tile([P, QW], F32, tag="w")
                for dc in range(DCH):
                    nc.tensor.matmul(
                        pv[:, :256],
                        xT4[:, j, dc, off : off + P],
                        wv_sb[:, dc, :],
                        start=(dc == 0),
                        stop=(dc == DCH - 1),
                    )
                nc.vector.tensor_scalar_mul(
                    ve4[:, st, :, DH : 2 * DH],
                    pv[:, :256].rearrange("p (h c) -> p h c", h=HPC),
                    maskp[:, st : st + 1],
                )

            def qk_proj(pr, qt):
                sl = slice(qt * QW, (qt + 1) * QW)
                for dst, w_sb, b_sb in ((QT, wq_sb, bq_sb), (KT, wk_sb, bk_sb)):
                    pq = ps_w.tile([P, QW], F32, tag="w")
                    for dc in range(DCH):
                        nc.tensor.matmul(
                            pq[:],
                            w_sb[:, dc, pr * P : (pr + 1) * P],
                            xT4[:, qt, dc, :],
                            start=(dc == 0),
                            stop=(dc == DCH - 1),
                        )
                    nc.vector.tensor_scalar_add(
                        dst[:, pr, sl], pq[:], b_sb[:, pr : pr + 1]
                    )

            att_state = {}

            def _emit_ctx(pr, qt, cps, et, kc):
                for hh in range(2):
                    h = 2 * pr + hh
                    nc.tensor.matmul(
                        cps[hh][:, :],
                        VE[:, kc, h * P : (h + 1) * P],
                        et[:, hh * QW : (hh + 1) * QW],
                        start=(kc == 0),
                        stop=(kc == SC_CHUNKS - 1),
                    )

            def attn_chunks(pr, qt, kcs):
                qsl = slice(qt * QW, (qt + 1) * QW)
                if (pr, qt) not in att_state:
                    att_state[(pr, qt)] = [
                        [
                            ps_ctx.tile([P, QW], F32, tag="ctx", name=f"ctx{hh}")
                            for hh in range(2)
                        ],
                        [],  # pending (et, kc) whose ctx is not yet emitted
                    ]
                st_ = att_state[(pr, qt)]
                cps, pending = st_
                for kc in kcs:
                    sc = ps_sc.tile([P, 2 * QW], F32, tag="sc")
                    for hh in range(2):
                        nc.tensor.matmul(
                            sc[:, hh * QW : (hh + 1) * QW],
                            KT[hh * DH : (hh + 1) * DH, pr, kc * P : (kc + 1) * P],
                            QT[hh * DH : (hh + 1) * DH, pr, qsl],
                            start=True,
                            stop=True,
                            tile_position=(hh * DH, 0),
                        )
                    et = ep.tile([P, 2 * QW], BF16, tag="et")
                    nc.scalar.activation(et[:], sc[:], AF.Exp, scale=float(SCALE))
                    # ctx lags CTX_LAG chunks so upcoming scores aren't stuck
                    # behind ctx's wait-for-exp in the PE's in-order stream
                    # (also covers the previous qtile's normalization latency
                    # at qt boundaries before ctx's PSUM slot is reusable)
                    pending.append((et, kc))
                    if len(pending) > CTX_LAG:
                        _emit_ctx(pr, qt, cps, *pending.pop(0))

            def attn_norm_fine(pr, qt):
                # last-tile tail: flush ctx, then normalize per 128-query
                # slice and issue each out_proj as soon as its slice is ready
                qsl0 = qt * QW
                cps, pending = att_state.pop((pr, qt))
                for p_ in pending:
                    _emit_ctx(pr, qt, cps, *p_)
                for i in range(4):
                    csl = slice(i * P, (i + 1) * P)
                    for hh in range(2):
                        denr = sp.tile([1, P], F32, tag="denrf", name=f"dnf{hh}")
                        nc.vector.reciprocal_approx_fast(
                            denr[:], cps[hh][0:1, csl]
                        )
                        recB = sp.tile([DH, P], F32, tag="recBf", name=f"rbf{hh}")
                        nc.gpsimd.partition_broadcast(recB[:], denr[:])
                        nc.vector.tensor_mul(
                            ctxT[hh * DH : (hh + 1) * DH, pr, qsl0 + i * P : qsl0 + (i + 1) * P],
                            cps[hh][DH : 2 * DH, csl],
                            recB[:],
                        )
                    out_proj(4 * qt + i)

            def attn_norm_fine(pr, qt):
                # last-tile tail: flush ctx, then normalize per 128-query
                # slice and issue each out_proj as soon as its slice is ready
                qsl0 = qt * QW
                cps, pending = att_state.pop((pr, qt))
                for p_ in pending:
                    _emit_ctx(pr, qt, cps, *p_)
                for i in range(4):
                    csl = slice(i * P, (i + 1) * P)
                    for hh in range(2):
                        denr = sp.tile([1, P], F32, tag="denrf", name=f"dnf{hh}")
                        nc.vector.reciprocal_approx_fast(
                            denr[:], cps[hh][0:1, csl]
                        )
                        recB = sp.tile([DH, P], F32, tag="recBf", name=f"rbf{hh}")
                        nc.gpsimd.partition_broadcast(recB[:], denr[:])
                        nc.vector.tensor_mul(
                            ctxT[hh * DH : (hh + 1) * DH, pr, qsl0 + i * P : qsl0 + (i + 1) * P],
                            cps[hh][DH : 2 * DH, csl],
                            recB[:],
                        )
                    out_proj(4 * qt + i)

            def attn_norm(pr, qt):
                qsl = slice(qt * QW, (qt + 1) * QW)
                cps, pending = att_state.pop((pr, qt))
                for p_ in pending:
                    _emit_ctx(pr, qt, cps, *p_)
                # normalize: recip(den@partition0) -> broadcast -> ctx*rec
                for hh in range(2):
                    denr = sp.tile([1, QW], F32, tag="denr", name=f"denr{hh}")
                    nc.vector.reciprocal_approx_fast(denr[:], cps[hh][0:1, :])
                    recB = sp.tile([DH, QW], F32, tag="recB", name=f"recB{hh}")
                    nc.gpsimd.partition_broadcast(recB[:], denr[:])
                    nc.vector.tensor_mul(
                        ctxT[hh * DH : (hh + 1) * DH, pr, qsl],
                        cps[hh][DH : 2 * DH, :],
                        recB[:],
                    )

            def out_proj(st):
                ob = op_.tile([P, D], BF16, tag="ob")
                for nt in range(2):
                    po = ps_w.tile([P, QW], F32, tag="w")
                    for pr in range(PAIRS):
                        nc.tensor.matmul(
                            po[:],
                            ctxT[:, pr, st * P : (st + 1) * P],
                            wo_sb[:, pr, nt * QW : (nt + 1) * QW],
                            start=(pr == 0),
                            stop=(pr == PAIRS - 1),
                        )
                    nc.vector.tensor_copy(ob[:, nt * QW : (nt + 1) * QW], po[:])
                    # each half streams out as soon as its evacuation lands,
                    # split across both DMA queues to halve drain latency
                    eng = nc.sync if nt == 0 else nc.gpsimd
                    eng.dma_start(
                        out[st * P : (st + 1) * P, nt * QW : (nt + 1) * QW],
                        ob[:, nt * QW : (nt + 1) * QW],
                    )

            def attention(pr, qt):
                attn_chunks(pr, qt, range(SC_CHUNKS))
                attn_norm(pr, qt)

            # ---- emission order (sets scheduling priority) ----
            # Chase: attention(0,{0,1}) kc-chunks interleave with the pair-0
            # QK / V projection chase so exp starts early and has 2 qtiles of
            # runway per x slice; later projections (pair 1, out) are emitted
            # one step late so the next scores outrank them and ACT never
            # starves.
            for j in range(QT_TILES):
                qk_proj(0, j)
                attn_chunks(0, 0, range(4 * j, 4 * j + 2))
                for st in range(4 * j, 4 * j + 4):
                    v_proj(st)
                attn_chunks(0, 0, range(4 * j + 2, 4 * j + 4))
            attn_norm(0, 0)
            attention(0, 1)
            qk_proj(1, 0)
            attention(0, 2)
            qk_proj(1, 1)
            attention(0, 3)
            qk_proj(1, 2)
            attn_chunks(1, 0, range(0, 8))
            qk_proj(1, 3)
            attn_chunks(1, 0, range(8, 16))
            attn_norm(1, 0)
            attention(1, 1)
            for st in range(0, 4):
                out_proj(st)
            attn_chunks(1, 2, range(0, 8))
            out_proj(4)
            out_proj(5)
            attn_chunks(1, 2, range(8, 16))
            attn_norm(1, 2)
            out_proj(6)
            out_proj(7)
            attn_chunks(1, 3, range(0, 6))
            out_proj(8)
            out_proj(9)
            attn_chunks(1, 3, range(6, 12))
            out_proj(10)
            out_proj(11)
            attn_chunks(1, 3, range(12, 16))
            attn_norm_fine(1, 3)

    nc.finalize()
    return nc


def ts(i, w):
    return slice(i * w, (i + 1) * w)


def _sb_w(w):
    """[D, n] weight slice -> SBUF layout [P, DCH, n]."""
    return np.ascontiguousarray(np.asarray(w).reshape(DCH, P, -1).transpose(1, 0, 2))


def shard_inputs(x, Wq, bq, Wk, bk, Wv, bv, Wo, bo, mask):
    """Full inputs -> list of 8 per-core input maps, pre-tiled to SBUF
    layouts (pure host-side layout prep; no kernel math moves to host)."""
    maskf = (~np.asarray(mask)).astype(np.float32)  # 1.0 = keep
    bf16 = ml_dtypes.bfloat16
    x = np.asarray(x, dtype=np.float32)
    # xt[p, j, c, s] = x[j*QW+s, c*P+p]
    xts = [
        np.ascontiguousarray(
            x[b].T.reshape(DCH, P, QT_TILES, QW).transpose(1, 2, 0, 3)
        ).astype(bf16)
        for b in range(2)
    ]
    Wqh = np.asarray(Wq, np.float32).astype(bf16)
    Wkh = np.asarray(Wk, np.float32).astype(bf16)
    Wvh = np.asarray(Wv, np.float32).astype(bf16)
    Woh = np.asarray(Wo, np.float32).astype(bf16)
    mask_t = [
        np.ascontiguousarray(maskf[b].reshape(SC_CHUNKS, P).T) for b in range(2)
    ]
    ins = []
    for c in range(N_CORES):
        b, g = divmod(c, 4)
        cs = slice(g * 256, (g + 1) * 256)
        wq_t = _sb_w(Wqh[:, cs])
        wk_t = _sb_w(Wkh[:, cs])
        ins.append(
            {
                "xt": xts[b],
                "wq0": np.ascontiguousarray(wq_t[:, :, 0:P]),
                "wq1": np.ascontiguousarray(wq_t[:, :, P:256]),
                "wk0": np.ascontiguousarray(wk_t[:, :, 0:P]),
                "wk1": np.ascontiguousarray(wk_t[:, :, P:256]),
                "wv": _sb_w(Wvh[:, cs]),
                "wo": np.ascontiguousarray(
                    Woh[cs, :].reshape(PAIRS, P, D).transpose(1, 0, 2)
                ),
                "bq": np.ascontiguousarray(
                    np.asarray(bq, np.float32)[cs].reshape(PAIRS, P).T
                ),
                "bk": np.ascontiguousarray(
                    np.asarray(bk, np.float32)[cs].reshape(PAIRS, P).T
                ),
                "maskf": mask_t[b],
            }
        )
    return ins


def gather_outputs(results, bv, Wo, bo):
    """8 per-core partial outputs -> full (2, S, D) fp32 output."""
    bo_eff = np.asarray(bo, dtype=np.float32) + np.asarray(
        bv, dtype=np.float32
    ) @ np.asarray(Wo, dtype=np.float32)
    outs = []
    for b in range(2):
        acc = results[4 * b]["out"].astype(np.float32).copy()
        for g in range(1, 4):
            acc += results[4 * b + g]["out"]
        outs.append(acc + bo_eff)
    return np.stack(outs, axis=0)


_NC_CACHE = []


def _get_nc():
    if not _NC_CACHE:
        _NC_CACHE.append(build())
    return _NC_CACHE[0]


def run_sharded(inputs, trace=False, tmpdir=None):
    """Shard, run on cores 0-7, gather. Returns (output, BassKernelResults)."""
    nc = _get_nc()
    ins = shard_inputs(**inputs)
    res = run_bass_kernel_spmd(
        nc, ins, core_ids=list(range(N_CORES)), trace=trace, tmpdir=tmpdir
    )
    full = gather_outputs(res.results, inputs["bv"], inputs["Wo"], inputs["bo"])
    return full, res


def kernel(**inputs) -> np.ndarray:
    full, _ = run_sharded(inputs, trace=False)
    return full
